# revision 1
# baseline (speedup 1.0000x reference)
"""GCN model (3x GCNConv + 2x BatchNorm + global mean pool + linear) on 8 TRN2 cores.

Strategy:
- Host: add self-loops as explicit edges; bin-pack nodes into 392 bins of <=128
  slots balancing per-bin edge counts; remap node ids to (bin, slot); assign 49
  bins per core (edges partitioned by dst bin); pad each bin's edge list to T
  tiles of 128 edges.
- Device (SPMD x8): GCNConv aggregation = indirect-DMA gather of 128 bf16
  feature rows + one-hot (iota compare) matmul accumulating into a PSUM tile
  per 128-dst-slot bin.  deg^-1/2 norms folded in as row pre-scale (on the
  gathered table) and per-partition post-scale.  conv bias added inside PSUM
  via a K=1 matmul of (1/dis) x b.  BatchNorms folded into the following
  matmul's weights (scale) + a rank-1 PSUM correction (shift).  Mean-pool via
  one-hot matmul.  Cross-core: bf16 AllGather of the node-feature table
  between layers, small AllReduduce for BN stats / pooled sums.
"""

import os

import numpy as np
import ml_dtypes

N = 50000
E = 800000
IN = 128
HID = 256
G = 64
NCLS = 10
BN_EPS = 1e-5

P = 128
CORES = 8
BINS = 392          # global 128-slot bins (392*128 = 50176 slots)
CH = BINS // CORES  # 49 bins per core
SL = CH * P         # 6272 slots per core
S = BINS * P        # 50176 total slots

F32 = np.float32
BF16 = ml_dtypes.bfloat16

LAST_EXEC_NS = None
LAST_RESULTS = None


def _preprocess(x, edge_index, batch):
    import heapq

    src = edge_index[0].astype(np.int64)
    dst = edge_index[1].astype(np.int64)
    deg = 1.0 + np.bincount(dst, minlength=N).astype(np.float64)
    dis = (1.0 / np.sqrt(deg)).astype(F32)
    invdis = np.sqrt(deg).astype(F32)

    # ---- bin-pack nodes into BINS bins (<=128 nodes each), balancing edges ----
    w = np.bincount(dst, minlength=N).astype(np.int64) + 1  # incl. self-loop
    order = np.argsort(-w, kind="stable")
    heap = [(0, b) for b in range(BINS)]
    heapq.heapify(heap)
    count = np.zeros(BINS, np.int64)
    new_id = np.empty(N, np.int64)
    for n in order:
        while True:
            load, b = heapq.heappop(heap)
            if count[b] < P:
                break
        new_id[n] = b * P + count[b]
        count[b] += 1
        heapq.heappush(heap, (load + int(w[n]), b))

    # ---- edges (with self-loops), grouped by dst bin, padded to tiles ----
    es = np.concatenate([src, np.arange(N, dtype=np.int64)])
    ed = np.concatenate([dst, np.arange(N, dtype=np.int64)])
    es_s = new_id[es]
    ed_s = new_id[ed]
    bin_e = ed_s // P
    o = np.argsort(bin_e, kind="stable")
    es_s, ed_s, bin_e = es_s[o], ed_s[o], bin_e[o]
    cnt_bin = np.bincount(bin_e, minlength=BINS)
    T = int(np.ceil(cnt_bin.max() / P))
    cap = T * P
    starts = np.concatenate([[0], np.cumsum(cnt_bin)[:-1]])
    rank = np.arange(len(es_s)) - starts[bin_e]
    pos = bin_e * cap + rank
    src_pad = np.zeros(BINS * cap, np.int32)
    dst_pad = np.full(BINS * cap, 255.0, F32)
    src_pad[pos] = es_s.astype(np.int32)
    dst_pad[pos] = (ed_s % P).astype(F32)
    src_tiles = src_pad.reshape(BINS * T, P)  # [tiles, 128]
    dst_tiles = dst_pad.reshape(BINS * T, P)

    # ---- per-slot arrays ----
    slot_dis = np.zeros(S, F32)
    slot_dis[new_id] = dis
    slot_invdis = np.zeros(S, F32)
    slot_invdis[new_id] = invdis
    slot_batch = np.full(S, 255.0, F32)
    slot_batch[new_id] = batch.astype(F32)
    xT = np.zeros((IN, S), F32)
    xT[:, new_id] = x.T
    xT = xT.astype(BF16)

    cnts = np.bincount(batch.astype(np.int64), minlength=G).astype(F32)
    cnts = np.maximum(cnts, 1.0)

    per_core = []
    for c in range(CORES):
        t0, t1 = c * CH * T, (c + 1) * CH * T
        s0, s1 = c * SL, (c + 1) * SL
        per_core.append(
            dict(
                srcidx=np.ascontiguousarray(src_tiles[t0:t1].T),       # [128, CH*T] i32
                dstloc=np.ascontiguousarray(dst_tiles[t0:t1].T),       # [128, CH*T] f32
                disloc=np.ascontiguousarray(slot_dis[s0:s1].reshape(CH, P).T),   # [128, CH]
                invdis=np.ascontiguousarray(slot_invdis[s0:s1].reshape(1, SL)),  # [1, SL]
                batchloc=np.ascontiguousarray(slot_batch[s0:s1].reshape(CH, P).T),  # [128, CH]
            )
        )

    shared = dict(
        xT=xT,
        disall=np.ascontiguousarray(slot_dis.reshape(BINS, P).T),  # [128, BINS]
        iota128=np.tile(np.arange(P, dtype=F32), (P, 1)),          # [128,128]
        eye128=np.eye(P, dtype=F32),
        onescol=np.ones((P, 1), F32),
        onesrow=np.ones((1, P), F32),
        cntrow=cnts.reshape(1, G),
        invcntcol=(1.0 / cnts).reshape(G, 1),
    )
    return per_core, shared, T


def _build(nc, tc, T):
    from concourse import bass, mybir
    STOP = os.environ.get('K_STOP', '')

    f32 = mybir.dt.float32
    bf16 = mybir.dt.bfloat16
    i32 = mybir.dt.int32
    AF = mybir.ActivationFunctionType
    OP = mybir.AluOpType
    NT = CH * T  # tiles per core per layer

    # ---------------- parameters ----------------
    def par(name, shape, dt):
        return nc.declare_dram_parameter(name, list(shape), dt, isOutput=False)

    xT_d = par("xT", (IN, S), bf16)
    srcidx_d = par("srcidx", (P, NT), i32)
    dstloc_d = par("dstloc", (P, NT), f32)
    disloc_d = par("disloc", (P, CH), f32)
    invdis_d = par("invdis", (1, SL), f32)
    batch_d = par("batchloc", (P, CH), f32)
    disall_d = par("disall", (P, BINS), f32)
    iota_d = par("iota128", (P, P), f32)
    eye_d = par("eye128", (P, P), f32)
    onescol_d = par("onescol", (P, 1), f32)
    onesrow_d = par("onesrow", (1, P), f32)
    cntrow_d = par("cntrow", (1, G), f32)
    invcnt_d = par("invcntcol", (G, 1), f32)
    W1_d = par("W1", (IN, HID), f32)
    W2_d = par("W2", (HID, HID), f32)
    W3_d = par("W3", (HID, HID), f32)
    Wf_d = par("Wf", (HID, NCLS), f32)
    b1_d = par("b1", (1, HID), f32)
    b2_d = par("b2", (1, HID), f32)
    b3_d = par("b3", (1, HID), f32)
    bf_d = par("bf", (1, NCLS), f32)
    g1_d = par("g1c", (P, 2), f32)
    be1_d = par("be1c", (P, 2), f32)
    g2_d = par("g2c", (P, 2), f32)
    be2_d = par("be2c", (P, 2), f32)
    out_d = nc.declare_dram_parameter("out", [G, NCLS], f32, isOutput=True)

    # ---------------- device DRAM ----------------
    hs1_d = nc.dram_tensor("hs1", [S, HID], bf16)
    hs2_d = nc.dram_tensor("hs2", [S, HID], bf16)
    hs3_d = nc.dram_tensor("hs3", [S, HID], bf16)
    hsloc2_d = nc.dram_tensor("hsloc2", [SL, HID], bf16)
    hsloc3_d = nc.dram_tensor("hsloc3", [SL, HID], bf16)
    st1_in = nc.dram_tensor("st1_in", [P, 4], f32)
    st1_out = nc.dram_tensor("st1_out", [P, 4], f32)
    ar2_in = nc.dram_tensor("ar2_in", [P, 132], f32)
    ar2_out = nc.dram_tensor("ar2_out", [P, 132], f32)

    GRP = [list(range(CORES))]

    # ---------------- resident SBUF ----------------
    import contextlib

    ctx = contextlib.ExitStack()
    res = ctx.enter_context(tc.tile_pool(name="res", bufs=1))
    psr = ctx.enter_context(tc.tile_pool(name="psr", bufs=1, space="PSUM"))

    hloc = res.tile([P, CH * HID], f32)       # resident node features [slot, feat]
    hT0 = res.tile([P, SL], bf16)             # transposed features, feat block 0
    hT1 = res.tile([P, SL], bf16)
    srcidx = res.tile([P, NT], i32)
    dstloc = res.tile([P, NT], f32)
    disloc = res.tile([P, CH], f32)
    invdis = res.tile([1, SL], f32)
    batchloc = res.tile([P, CH], f32)
    disall = res.tile([P, BINS], f32)
    iota = res.tile([P, P], f32)
    eye = res.tile([P, P], f32)
    onescol = res.tile([P, 1], f32)
    onesrow = res.tile([1, P], f32)
    cntrow = res.tile([1, G], f32)
    invcnt = res.tile([G, 1], f32)
    W1 = res.tile([IN, HID], bf16)
    W2s = [res.tile([P, HID], bf16, tag=f"w2_{f}", name=f"w2_{f}") for f in range(2)]
    W3s = [res.tile([P, HID], f32, tag=f"w3_{f}", name=f"w3_{f}") for f in range(2)]
    W3p = [res.tile([P, HID], bf16, tag=f"w3p_{f}", name=f"w3p_{f}") for f in range(2)]
    Wfs = [res.tile([P, NCLS], f32, tag=f"wf_{f}", name=f"wf_{f}") for f in range(2)]
    Wfp = [res.tile([P, NCLS], f32, tag=f"wfp_{f}", name=f"wfp_{f}") for f in range(2)]
    b1 = res.tile([1, HID], f32)
    b2 = res.tile([1, HID], f32)
    b3 = res.tile([1, HID], f32)
    bfr = res.tile([1, NCLS], f32)
    g1c = res.tile([P, 2], f32)
    be1c = res.tile([P, 2], f32)
    g2c = res.tile([P, 2], f32)
    be2c = res.tile([P, 2], f32)
    rrow = res.tile([1, HID], f32)
    bfp = res.tile([1, NCLS], f32)
    scale1 = res.tile([P, 2], f32)
    shift1 = res.tile([P, 2], f32)
    scale2 = res.tile([P, 2], f32)
    shift2 = res.tile([P, 2], f32)
    stats1 = res.tile([P, 4], f32)
    ar2 = res.tile([P, 132], f32)
    epscol = res.tile([P, 1], f32)
    nc.vector.memset(epscol[:], BN_EPS)

    dma = nc.sync.dma_start
    for dst_t, src_t in [
        (srcidx, srcidx_d), (dstloc, dstloc_d), (disloc, disloc_d),
        (invdis, invdis_d), (batchloc, batch_d), (disall, disall_d),
        (iota, iota_d), (eye, eye_d), (onescol, onescol_d),
        (onesrow, onesrow_d), (cntrow, cntrow_d), (invcnt, invcnt_d),
        (b1, b1_d), (b2, b2_d), (b3, b3_d), (bfr, bf_d),
        (g1c, g1_d), (be1c, be1_d), (g2c, g2_d), (be2c, be2_d),
    ]:
        dma(out=dst_t[:], in_=src_t[:, :])
    # weights: cast f32 -> bf16 through SBUF
    wtmp_pool = tc.alloc_tile_pool(name="wtmp", bufs=2)
    wt = wtmp_pool.tile([IN, HID], f32, tag="wt")
    dma(out=wt[:], in_=W1_d[:, :])
    nc.vector.tensor_copy(out=W1[:], in_=wt[:])
    for f in range(2):
        wt2 = wtmp_pool.tile([P, HID], f32, tag="wt")
        dma(out=wt2[:], in_=W2_d[f * P:(f + 1) * P, :])
        nc.vector.tensor_copy(out=W2s[f][:], in_=wt2[:])
        dma(out=W3s[f][:], in_=W3_d[f * P:(f + 1) * P, :])
        dma(out=Wfs[f][:], in_=Wf_d[f * P:(f + 1) * P, :])
    wtmp_pool.release()


    def _early_out(tag):
        with tc.tile_pool(name="eo_" + tag, bufs=1) as eo:
            z = eo.tile([G, NCLS], f32, tag="z", name="z_" + tag)
            nc.vector.tensor_copy(out=z[:], in_=hloc[0:G, 0:NCLS])
            dma(out=out_d[:, :], in_=z[:])
    # ================= phase A: hs1 = dis * (x @ W1), all slots =================
    with tc.tile_pool(name="pA", bufs=4) as pA, \
         tc.tile_pool(name="pAp", bufs=2, space="PSUM") as pAp:
        for g in range(BINS):
            xt = pA.tile([P, P], bf16, tag="xt")
            dma(out=xt[:], in_=xT_d[:, g * P:(g + 1) * P])
            ps = pAp.tile([P, HID], f32, tag="ps")
            nc.tensor.matmul(out=ps[:], lhsT=xt[:], rhs=W1[:], start=True, stop=True)
            hs = pA.tile([P, HID], bf16, tag="hs")
            nc.scalar.activation(out=hs[:], in_=ps[:], func=AF.Copy,
                                 scale=disall[:, g:g + 1])
            dma(out=hs1_d[g * P:(g + 1) * P, :], in_=hs[:])

    if STOP == 'A':
        _early_out('A'); ctx.close(); return

    # ================= edge aggregation pass =================
    def edge_pass(hs_table, bias_row, out_getter):
        """out_getter(j) -> SBUF AP [128, HID] destination for relu'd result."""
        with tc.tile_pool(name="pE", bufs=6) as pE, \
             tc.tile_pool(name="pEp", bufs=2, space="PSUM") as pEp:
            for j in range(CH):
                ps = pEp.tile([P, HID], f32, tag="agg")
                for t in range(T):
                    ti = j * T + t
                    gt = pE.tile([P, HID], bf16, tag="gath")
                    nc.gpsimd.indirect_dma_start(
                        out=gt[:], out_offset=None,
                        in_=hs_table[:, :],
                        in_offset=bass.IndirectOffsetOnAxis(
                            ap=srcidx[:, ti:ti + 1], axis=0),
                    )
                    oh = pE.tile([P, P], bf16, tag="oh")
                    nc.vector.tensor_tensor(
                        out=oh[:], in0=dstloc[:, ti:ti + 1].to_broadcast([P, P]),
                        in1=iota[:], op=OP.is_equal)
                    nc.tensor.matmul(out=ps[:], lhsT=oh[:], rhs=gt[:],
                                     start=(t == 0), stop=False)
                # + (1/dis) x bias  (K=1 rank-1 update), then relu(dis * psum)
                nc.tensor.matmul(out=ps[:], lhsT=invdis[0:1, j * P:(j + 1) * P],
                                 rhs=bias_row[:], start=False, stop=True)
                nc.scalar.activation(out=out_getter(j), in_=ps[:], func=AF.Relu,
                                     scale=disloc[:, j:j + 1])

    def hchunk(j):
        return hloc[:, j * HID:(j + 1) * HID]

    edge_pass(hs1_d, b1, hchunk)
    if STOP == 'E1':
        _early_out('E1'); ctx.close(); return

    # ============== transpose hloc -> hT (bf16) ==============
    def transpose_h():
        with tc.tile_pool(name="pT", bufs=4) as pT, \
             tc.tile_pool(name="pTp", bufs=4, space="PSUM") as pTp:
            for j in range(CH):
                for f, hT in enumerate((hT0, hT1)):
                    pst = pTp.tile([P, P], f32, tag="pst")
                    nc.tensor.transpose(
                        out=pst[:], in_=hloc[:, j * HID + f * P: j * HID + (f + 1) * P],
                        identity=eye[:])
                    nc.scalar.activation(out=hT[:, j * P:(j + 1) * P], in_=pst[:],
                                         func=AF.Copy)

    # ============== hs_next = dis * (h @ W) [+ dis x r], allgather ==============
    def make_hs(Ws, hsloc_dram, hs_dram, add_r):
        with tc.tile_pool(name="pH", bufs=4) as pH, \
             tc.tile_pool(name="pHp", bufs=2, space="PSUM") as pHp:
            for j in range(CH):
                ps = pHp.tile([P, HID], f32, tag="hs")
                nc.tensor.matmul(out=ps[:], lhsT=hT0[:, j * P:(j + 1) * P],
                                 rhs=Ws[0][:], start=True, stop=False)
                nc.tensor.matmul(out=ps[:], lhsT=hT1[:, j * P:(j + 1) * P],
                                 rhs=Ws[1][:], start=False, stop=not add_r)
                if add_r:
                    nc.tensor.matmul(out=ps[:], lhsT=onesrow[:],
                                     rhs=rrow[:], start=False, stop=True)
                hst = pH.tile([P, HID], bf16, tag="hst")
                nc.scalar.activation(out=hst[:], in_=ps[:], func=AF.Copy,
                                     scale=disloc[:, j:j + 1])
                dma(out=hsloc_dram[j * P:(j + 1) * P, :], in_=hst[:])
        nc.gpsimd.collective_compute(
            "AllGather", mybir.AluOpType.bypass, replica_groups=GRP,
            ins=[hsloc_dram.ap().opt()], outs=[hs_dram.ap().opt()])

    transpose_h()
    if STOP == 'T1':
        _early_out('T1'); ctx.close(); return
    make_hs(W2s, hsloc2_d, hs2_d, add_r=False)
    if STOP == 'H2':
        _early_out('H2'); ctx.close(); return
    edge_pass(hs2_d, b2, hchunk)
    if STOP == 'E2':
        _early_out('E2'); ctx.close(); return

    # ============== BN1 stats -> allreduce -> scale1/shift1 ==============
    with tc.tile_pool(name="pS", bufs=4) as pS, \
         tc.tile_pool(name="pSp", bufs=1, space="PSUM") as pSp:
        s1 = [pSp.tile([P, 1], f32, tag=f"s1_{f}", name=f"s1_{f}") for f in range(2)]
        s2 = [pSp.tile([P, 1], f32, tag=f"s2_{f}", name=f"s2_{f}") for f in range(2)]
        for j in range(CH):
            sq = pS.tile([P, HID], f32, tag="sq")
            nc.scalar.activation(out=sq[:], in_=hchunk(j), func=AF.Square)
            for f in range(2):
                hsl = hloc[:, j * HID + f * P: j * HID + (f + 1) * P]
                nc.tensor.matmul(out=s1[f][:], lhsT=hsl, rhs=onescol[:],
                                 start=(j == 0), stop=(j == CH - 1))
                nc.tensor.matmul(out=s2[f][:], lhsT=sq[:, f * P:(f + 1) * P],
                                 rhs=onescol[:], start=(j == 0), stop=(j == CH - 1))
        st = pS.tile([P, 4], f32, tag="st")
        for f in range(2):
            nc.vector.tensor_copy(out=st[:, f:f + 1], in_=s1[f][:])
            nc.vector.tensor_copy(out=st[:, 2 + f:3 + f], in_=s2[f][:])
        dma(out=st1_in[:, :], in_=st[:])
    nc.gpsimd.collective_compute(
        "AllReduce", mybir.AluOpType.add, replica_groups=GRP,
        ins=[st1_in.ap().opt()], outs=[st1_out.ap().opt()])
    dma(out=stats1[:], in_=st1_out[:, :])

    def bn_fold(stats_sums, stats_sqs, gc, bec, scale_t, shift_t, pool):
        """stats cols -> scale/shift [P,2] (feature-column layout)."""
        mu = pool.tile([P, 2], f32, tag="mu")
        var = pool.tile([P, 2], f32, tag="var")
        tmp = pool.tile([P, 2], f32, tag="tmp")
        nc.vector.tensor_scalar_mul(out=mu[:], in0=stats_sums, scalar1=1.0 / N)
        nc.vector.tensor_scalar_mul(out=var[:], in0=stats_sqs, scalar1=1.0 / N)
        nc.vector.tensor_tensor(out=tmp[:], in0=mu[:], in1=mu[:], op=OP.mult)
        nc.vector.tensor_tensor(out=var[:], in0=var[:], in1=tmp[:], op=OP.subtract)
        nc.scalar.activation(out=tmp[:], in_=var[:], func=AF.Sqrt, bias=epscol[:])
        nc.vector.reciprocal(out=tmp[:], in_=tmp[:])
        nc.vector.tensor_tensor(out=scale_t[:], in0=gc[:], in1=tmp[:], op=OP.mult)
        nc.vector.tensor_tensor(out=tmp[:], in0=mu[:], in1=scale_t[:], op=OP.mult)
        nc.vector.tensor_tensor(out=shift_t[:], in0=bec[:], in1=tmp[:], op=OP.subtract)

    with tc.tile_pool(name="pB", bufs=1) as pB, \
         tc.tile_pool(name="pBp", bufs=1, space="PSUM") as pBp:
        bn_fold(stats1[:, 0:2], stats1[:, 2:4], g1c, be1c, scale1, shift1, pB)
        # W3' = scale1 (*) W3 rows; rrow = shift1 @ W3
        psr_ = pBp.tile([1, HID], f32, tag="rr")
        for f in range(2):
            w3f = pB.tile([P, HID], f32, tag="w3f")
            nc.vector.tensor_scalar_mul(out=w3f[:], in0=W3s[f][:],
                                        scalar1=scale1[:, f:f + 1])
            nc.vector.tensor_copy(out=W3p[f][:], in_=w3f[:])
            nc.tensor.matmul(out=psr_[:], lhsT=shift1[:, f:f + 1], rhs=W3s[f][:],
                             start=(f == 0), stop=(f == 1))
        nc.vector.tensor_copy(out=rrow[:], in_=psr_[:])

    if STOP == 'B1':
        _early_out('B1'); ctx.close(); return
    transpose_h()
    make_hs(W3p, hsloc3_d, hs3_d, add_r=True)
    if STOP == 'H3':
        _early_out('H3'); ctx.close(); return
    edge_pass(hs3_d, b3, hchunk)
    if STOP == 'E3':
        _early_out('E3'); ctx.close(); return

    # ====== BN2 stats + pooled sums -> one allreduce ======
    with tc.tile_pool(name="pG", bufs=4) as pG, \
         tc.tile_pool(name="pGp", bufs=1, space="PSUM") as pGp:
        pool_ps = [pGp.tile([P, G], f32, tag=f"pool_{f}", name=f"pool_{f}") for f in range(2)]
        s1 = [pGp.tile([P, 1], f32, tag=f"gs1_{f}", name=f"gs1_{f}") for f in range(2)]
        s2 = [pGp.tile([P, 1], f32, tag=f"gs2_{f}", name=f"gs2_{f}") for f in range(2)]
        for j in range(CH):
            sq = pG.tile([P, HID], f32, tag="sq")
            nc.scalar.activation(out=sq[:], in_=hchunk(j), func=AF.Square)
            ohp = pG.tile([P, G], f32, tag="ohp")
            nc.vector.tensor_tensor(
                out=ohp[:], in0=batchloc[:, j:j + 1].to_broadcast([P, G]),
                in1=iota[:, 0:G], op=OP.is_equal)
            for f in range(2):
                hsl = hloc[:, j * HID + f * P: j * HID + (f + 1) * P]
                nc.tensor.matmul(out=pool_ps[f][:], lhsT=hsl, rhs=ohp[:],
                                 start=(j == 0), stop=(j == CH - 1))
                nc.tensor.matmul(out=s1[f][:], lhsT=hsl, rhs=onescol[:],
                                 start=(j == 0), stop=(j == CH - 1))
                nc.tensor.matmul(out=s2[f][:], lhsT=sq[:, f * P:(f + 1) * P],
                                 rhs=onescol[:], start=(j == 0), stop=(j == CH - 1))
        arp = pG.tile([P, 132], f32, tag="arp")
        for f in range(2):
            nc.vector.tensor_copy(out=arp[:, f * G:(f + 1) * G], in_=pool_ps[f][:])
            nc.vector.tensor_copy(out=arp[:, 128 + f:129 + f], in_=s1[f][:])
            nc.vector.tensor_copy(out=arp[:, 130 + f:131 + f], in_=s2[f][:])
        dma(out=ar2_in[:, :], in_=arp[:])
    nc.gpsimd.collective_compute(
        "AllReduce", mybir.AluOpType.add, replica_groups=GRP,
        ins=[ar2_in.ap().opt()], outs=[ar2_out.ap().opt()])
    dma(out=ar2[:], in_=ar2_out[:, :])

    # ====== fold BN2 into Wf, final matmul ======
    with tc.tile_pool(name="pF", bufs=1) as pF, \
         tc.tile_pool(name="pFp", bufs=1, space="PSUM") as pFp:
        bn_fold(ar2[:, 128:130], ar2[:, 130:132], g2c, be2c, scale2, shift2, pF)
        psb = pFp.tile([1, NCLS], f32, tag="psb")
        for f in range(2):
            nc.vector.tensor_scalar_mul(out=Wfp[f][:], in0=Wfs[f][:],
                                        scalar1=scale2[:, f:f + 1])
            nc.tensor.matmul(out=psb[:], lhsT=shift2[:, f:f + 1], rhs=Wfs[f][:],
                             start=(f == 0), stop=False)
        nc.tensor.matmul(out=psb[:], lhsT=onesrow[0:1, 0:1], rhs=bfr[:],
                         start=False, stop=True)
        nc.vector.tensor_copy(out=bfp[:], in_=psb[:])

        pso = pFp.tile([G, NCLS], f32, tag="pso")
        for f in range(2):
            nc.tensor.matmul(out=pso[:], lhsT=ar2[:, f * G:(f + 1) * G],
                             rhs=Wfp[f][:], start=(f == 0), stop=False)
        nc.tensor.matmul(out=pso[:], lhsT=cntrow[:], rhs=bfp[:],
                         start=False, stop=True)
        osb = pF.tile([G, NCLS], f32, tag="osb")
        nc.vector.tensor_scalar_mul(out=osb[:], in0=pso[:], scalar1=invcnt[:])
        dma(out=out_d[:, :], in_=osb[:])

    ctx.close()


def kernel(x, edge_index, batch, W1, b1, W2, b2, W3, b3, g1, be1, g2, be2, Wf, bf):
    global LAST_EXEC_NS, LAST_RESULTS
    from concourse import bacc, tile
    from concourse.bass_utils import run_bass_kernel_spmd

    x = np.asarray(x)
    edge_index = np.asarray(edge_index)
    batch = np.asarray(batch)

    per_core, shared, T = _preprocess(x, edge_index, batch)

    nc = bacc.Bacc("TRN2", target_bir_lowering=False, debug=False,
                   num_devices=CORES)
    with tile.TileContext(nc) as tc:
        _build(nc, tc, T)
    nc.compile()

    def col2(v):
        return np.ascontiguousarray(np.asarray(v, F32).reshape(2, P).T)

    base = dict(
        xT=shared["xT"], disall=shared["disall"].astype(F32),
        iota128=shared["iota128"], eye128=shared["eye128"],
        onescol=shared["onescol"], onesrow=shared["onesrow"],
        cntrow=shared["cntrow"], invcntcol=shared["invcntcol"],
        W1=np.asarray(W1, F32), W2=np.asarray(W2, F32), W3=np.asarray(W3, F32),
        Wf=np.asarray(Wf, F32),
        b1=np.asarray(b1, F32).reshape(1, HID), b2=np.asarray(b2, F32).reshape(1, HID),
        b3=np.asarray(b3, F32).reshape(1, HID), bf=np.asarray(bf, F32).reshape(1, NCLS),
        g1c=col2(g1), be1c=col2(be1), g2c=col2(g2), be2c=col2(be2),
    )
    in_maps = []
    for c in range(CORES):
        m = dict(base)
        m.update(per_core[c])
        in_maps.append(m)

    res = run_bass_kernel_spmd(nc, in_maps, core_ids=list(range(CORES)))
    LAST_EXEC_NS = res.exec_time_ns
    LAST_RESULTS = res
    return np.asarray(res.results[0]["out"], F32)



# revision 9
# speedup vs baseline: 1.5293x; 1.5293x over previous
"""GCN model (3x GCNConv + 2x BatchNorm + global mean pool + linear) on 8 TRN2 cores.

Strategy (v2):
- Host: bin-pack nodes into 392 bins of <=128 slots balancing per-bin in-edge
  counts; remap node ids to (bin, slot); 49 bins per core.  Per bin, edges are
  split by src slot into LOW (<32768) and HIGH streams so gather indices fit
  int16; each segment padded to 128-edge tiles with fixed per-bin tile counts
  (TL/TH = global maxima) so the SPMD instruction stream is core-uniform.
- Device (SPMD x8): edge aggregation via batched SWDGE dma_gather (1024 rows
  per instruction, 8x fewer fixed-overhead hits than 128-row indirect DMA)
  + one-hot (iota compare, 4 tiles per DVE op) matmuls accumulating in PSUM.
- Conv1 is computed as (A x) @ W1: gathers 128-dim dis-scaled x rows (host
  param), accumulates transposed so hT is produced with no transposes and no
  device-side x@W1 pre-pass.
- Self-loop contributions added via eye-matmul on resident local rows (not
  gathered).  Conv biases as K=1 rank-1 matmuls into PSUM.  BatchNorms folded
  into following matmul weights + rank-1 shift.  Cross-core: bf16 AllGather
  (Shared outputs) of the node-feature table between layers, small AllReduce
  for BN stats / pooled sums.
"""

import os

import numpy as np
import ml_dtypes

N = 50000
E = 800000
IN = 128
HID = 256
G = 64
NCLS = 10
BN_EPS = 1e-5

P = 128
CORES = 8
BINS = 392          # global 128-slot bins (392*128 = 50176 slots)
CH = BINS // CORES  # 49 bins per core
SL = CH * P         # 6272 slots per core
S = BINS * P        # 50176 total slots
HALF = 32768        # int16 index limit for gather tables
CT = 8              # tiles per gather chunk (1024 idxs = 64 desc/engine cap)

F32 = np.float32
BF16 = ml_dtypes.bfloat16

LAST_EXEC_NS = None
LAST_RESULTS = None


def _wrap16(idx):
    """[n] int -> [128, n//16] int16: element e at [e%16, e//16], replicated x8."""
    n = len(idx)
    a = np.asarray(idx, np.int16).reshape(n // 16, 16).T
    return np.ascontiguousarray(np.tile(a, (8, 1)))


def _preprocess(x, edge_index, batch):
    import heapq

    src = edge_index[0].astype(np.int64)
    dst = edge_index[1].astype(np.int64)
    deg = 1.0 + np.bincount(dst, minlength=N).astype(np.float64)
    dis = (1.0 / np.sqrt(deg)).astype(F32)

    # ---- bin-pack nodes into BINS bins (<=128 nodes each), balancing edges ----
    w = np.bincount(dst, minlength=N).astype(np.int64)
    order = np.argsort(-w, kind="stable")
    heap = [(0, b) for b in range(BINS)]
    heapq.heapify(heap)
    count = np.zeros(BINS, np.int64)
    new_id = np.empty(N, np.int64)
    for n in order:
        while True:
            load, b = heapq.heappop(heap)
            if count[b] < P:
                break
        new_id[n] = b * P + count[b]
        count[b] += 1
        heapq.heappush(heap, (load + int(w[n]), b))

    # ---- edges grouped by dst bin, split by src half, sorted by src ----
    es = new_id[src]
    ed = new_id[dst]
    bin_e = ed // P
    # order: (bin, is_high, src)
    key = bin_e * 4 * S + (es >= HALF) * 2 * S + es
    o = np.argsort(key, kind="stable")
    es, ed, bin_e = es[o], ed[o], bin_e[o]
    islo = es < HALF
    nlo = np.bincount(bin_e[islo], minlength=BINS)
    nhi = np.bincount(bin_e[~islo], minlength=BINS)
    TL = int(np.ceil(nlo.max() / P))
    TH = int(np.ceil(nhi.max() / P))

    # per-bin padded segments
    capL, capH = TL * P, TH * P
    idxL = np.zeros((BINS, capL), np.int64)
    dstL = np.full((BINS, capL), 255.0, F32)
    idxH = np.zeros((BINS, capH), np.int64)
    dstH = np.full((BINS, capH), 255.0, F32)
    starts = np.concatenate([[0], np.cumsum(nlo + nhi)[:-1]])
    for b in range(BINS):
        s0 = starts[b]
        lo_n, hi_n = nlo[b], nhi[b]
        idxL[b, :lo_n] = es[s0:s0 + lo_n]
        dstL[b, :lo_n] = (ed[s0:s0 + lo_n] % P).astype(F32)
        idxH[b, :hi_n] = es[s0 + lo_n:s0 + lo_n + hi_n] - HALF
        dstH[b, :hi_n] = (ed[s0 + lo_n:s0 + lo_n + hi_n] % P).astype(F32)

    # ---- per-slot arrays ----
    slot_dis = np.zeros(S, F32)
    slot_dis[new_id] = dis
    slot_invdis = np.zeros(S, F32)
    slot_invdis[new_id] = np.sqrt(deg).astype(F32)
    slot_mask = np.zeros(S, F32)
    slot_mask[new_id] = 1.0
    slot_batch = np.full(S, 255.0, F32)
    slot_batch[new_id] = batch.astype(F32)

    # dis-scaled x rows, slot order: gather table for pass 1
    xs = np.zeros((S, IN), F32)
    xs[new_id] = x * dis[:, None]
    xs = xs.astype(BF16)

    cnts = np.bincount(batch.astype(np.int64), minlength=G).astype(F32)
    cnts = np.maximum(cnts, 1.0)

    per_core = []
    for c in range(CORES):
        b0, b1 = c * CH, (c + 1) * CH
        s0, s1 = c * SL, (c + 1) * SL
        per_core.append(
            dict(
                idxL=_wrap16(idxL[b0:b1].reshape(-1)),        # [128, CH*TL*8]
                idxH=_wrap16(idxH[b0:b1].reshape(-1)),
                dstL=np.ascontiguousarray(
                    dstL[b0:b1].reshape(CH * TL, P).T).astype(BF16),  # [128, CH*TL]
                dstH=np.ascontiguousarray(
                    dstH[b0:b1].reshape(CH * TH, P).T).astype(BF16),
                disloc=np.ascontiguousarray(slot_dis[s0:s1].reshape(CH, P).T),  # [128, CH] f32
                disrow=np.ascontiguousarray(slot_dis[s0:s1].reshape(1, SL)),    # [1, SL] f32
                invdis=np.ascontiguousarray(slot_invdis[s0:s1].reshape(1, SL)).astype(BF16),
                maskrow=np.ascontiguousarray(slot_mask[s0:s1].reshape(1, SL)).astype(BF16),
                batchloc=np.ascontiguousarray(slot_batch[s0:s1].reshape(CH, P).T).astype(BF16),
                xsT=np.ascontiguousarray(xs[s0:s1].astype(F32).T).astype(BF16),  # [128, SL]
                disb=np.ascontiguousarray(
                    np.tile(slot_dis[s0:s1].reshape(1, SL), (P, 1))),  # [128, SL] f32
            )
        )

    iota4 = np.tile(np.arange(P, dtype=F32), (P, 4, 1)).astype(BF16)  # [128,4,128]
    iotaG = np.tile(np.arange(G, dtype=F32), (P, 1)).astype(BF16)     # [128,64]
    shared = dict(
        xs=xs,
        iota4=iota4,
        iotaG=iotaG,
        eye=np.eye(P, dtype=F32).astype(BF16),
        onescol=np.ones((P, 1), F32),
        onesrow=np.ones((1, P), F32).astype(BF16),
        cntrow=cnts.reshape(1, G),
        invcntcol=(1.0 / cnts).reshape(G, 1),
    )
    return per_core, shared, TL, TH


def _build(nc, tc, TL, TH):
    from concourse import bass, mybir
    from concourse import library_config
    STOP = os.environ.get('K_STOP', '')

    f32 = mybir.dt.float32
    bf16 = mybir.dt.bfloat16
    i16 = mybir.dt.int16
    AF = mybir.ActivationFunctionType
    OP = mybir.AluOpType

    NTL, NTH = CH * TL, CH * TH      # stream tiles per core per pass
    NCL = (NTL + CT - 1) // CT       # L chunks
    NCH = (NTH + CT - 1) // CT

    # ---------------- parameters ----------------
    def par(name, shape, dt):
        return nc.declare_dram_parameter(name, list(shape), dt, isOutput=False)

    xs_d = par("xs", (S, IN), bf16)
    idxL_d = par("idxL", (P, NTL * P // 16), i16)
    idxH_d = par("idxH", (P, NTH * P // 16), i16)
    dstL_d = par("dstL", (P, NTL), bf16)
    dstH_d = par("dstH", (P, NTH), bf16)
    disloc_d = par("disloc", (P, CH), f32)
    disrow_d = par("disrow", (1, SL), f32)
    invdis_d = par("invdis", (1, SL), bf16)
    maskrow_d = par("maskrow", (1, SL), bf16)
    batch_d = par("batchloc", (P, CH), bf16)
    xsT_d = par("xsT", (P, SL), bf16)
    disb_d = par("disb", (P, SL), f32)
    iota4_d = par("iota4", (P, 4, P), bf16)
    iotaG_d = par("iotaG", (P, G), bf16)
    eye_d = par("eye", (P, P), bf16)
    onescol_d = par("onescol", (P, 1), f32)
    onesrow_d = par("onesrow", (1, P), bf16)
    cntrow_d = par("cntrow", (1, G), f32)
    invcnt_d = par("invcntcol", (G, 1), f32)
    W1_d = par("W1", (IN, HID), f32)
    W2_d = par("W2", (HID, HID), f32)
    W3_d = par("W3", (HID, HID), f32)
    Wf_d = par("Wf", (HID, NCLS), f32)
    b1_d = par("b1", (1, HID), f32)
    b2_d = par("b2", (1, HID), f32)
    b3_d = par("b3", (1, HID), f32)
    bf_d = par("bf", (1, NCLS), f32)
    g1_d = par("g1c", (P, 2), f32)
    be1_d = par("be1c", (P, 2), f32)
    g2_d = par("g2c", (P, 2), f32)
    be2_d = par("be2c", (P, 2), f32)
    out_d = nc.declare_dram_parameter("out", [G, NCLS], f32, isOutput=True)

    # ---------------- device DRAM ----------------
    hs2loc_d = nc.dram_tensor("hs2loc", [SL, HID], bf16)
    hs3loc_d = nc.dram_tensor("hs3loc", [SL, HID], bf16)
    hs2_d = nc.dram_tensor("hs2", [S, HID], bf16, addr_space="Shared")
    hs3_d = nc.dram_tensor("hs3", [S, HID], bf16, addr_space="Shared")
    st1_in = nc.dram_tensor("st1_in", [P, 4], f32)
    st1_out = nc.dram_tensor("st1_out", [P, 4], f32, addr_space="Shared")
    ar2_in = nc.dram_tensor("ar2_in", [P, 132], f32)
    ar2_out = nc.dram_tensor("ar2_out", [P, 132], f32, addr_space="Shared")

    GRP = [list(range(CORES))]

    nc.gpsimd.load_library(library_config.mlp)

    # ---------------- resident SBUF ----------------
    import contextlib

    ctx = contextlib.ExitStack()
    res = ctx.enter_context(tc.tile_pool(name="res", bufs=1))

    idxL = res.tile([P, NTL * P // 16], i16)
    idxH = res.tile([P, NTH * P // 16], i16)
    dstL = res.tile([P, NTL], bf16)
    dstH = res.tile([P, NTH], bf16)
    disloc = res.tile([P, CH], f32)
    disrow = res.tile([1, SL], f32)
    invdis = res.tile([1, SL], bf16)
    maskrow = res.tile([1, SL], bf16)
    batchloc = res.tile([P, CH], bf16)
    iota4 = res.tile([P, 4, P], bf16)
    iotaG = res.tile([P, G], bf16)
    eye = res.tile([P, P], bf16)
    onescol = res.tile([P, 1], f32)
    onesrow = res.tile([1, P], bf16)
    cntrow = res.tile([1, G], f32)
    invcnt = res.tile([G, 1], f32)
    hT0 = res.tile([P, SL], bf16)             # transposed features, feat 0-127
    hT1 = res.tile([P, SL], bf16)             # feat 128-255
    hsloc = res.tile([P, CH * HID], bf16)     # resident local hs rows (self-loop)
    W1b = res.tile([IN, HID], bf16)
    W2s = [res.tile([P, HID], bf16, tag=f"w2_{f}", name=f"w2_{f}") for f in range(2)]
    W3s = [res.tile([P, HID], f32, tag=f"w3_{f}", name=f"w3_{f}") for f in range(2)]
    W3p = [res.tile([P, HID], bf16, tag=f"w3p_{f}", name=f"w3p_{f}") for f in range(2)]
    Wfs = [res.tile([P, NCLS], f32, tag=f"wf_{f}", name=f"wf_{f}") for f in range(2)]
    Wfp = [res.tile([P, NCLS], f32, tag=f"wfp_{f}", name=f"wfp_{f}") for f in range(2)]
    b1r = res.tile([1, HID], bf16)
    b2r = res.tile([1, HID], bf16)
    b3r = res.tile([1, HID], bf16)
    bfr = res.tile([1, NCLS], f32)
    g1c = res.tile([P, 2], f32)
    be1c = res.tile([P, 2], f32)
    g2c = res.tile([P, 2], f32)
    be2c = res.tile([P, 2], f32)
    rrow = res.tile([1, HID], bf16)
    bfp = res.tile([1, NCLS], f32)
    scale1 = res.tile([P, 2], f32)
    shift1 = res.tile([P, 2], f32)
    scale2 = res.tile([P, 2], f32)
    shift2 = res.tile([P, 2], f32)
    stats1 = res.tile([P, 4], f32)
    ar2 = res.tile([P, 132], f32)
    epscol = res.tile([P, 1], f32)
    nc.vector.memset(epscol[:], BN_EPS)

    dma = nc.sync.dma_start
    for dst_t, src_t in [
        (idxL, idxL_d), (idxH, idxH_d), (dstL, dstL_d), (dstH, dstH_d),
        (disloc, disloc_d), (disrow, disrow_d), (invdis, invdis_d),
        (maskrow, maskrow_d), (batchloc, batch_d),
        (iotaG, iotaG_d), (eye, eye_d), (onescol, onescol_d),
        (onesrow, onesrow_d), (cntrow, cntrow_d), (invcnt, invcnt_d),
        (bfr, bf_d), (g1c, g1_d), (be1c, be1_d), (g2c, g2_d), (be2c, be2_d),
    ]:
        dma(out=dst_t[:], in_=src_t[:, :])
    dma(out=iota4[:, :, :], in_=iota4_d[:, :, :])
    # weights / biases: cast f32 -> bf16 through SBUF
    wtmp_pool = tc.alloc_tile_pool(name="wtmp", bufs=2)
    wt = wtmp_pool.tile([IN, HID], f32, tag="wt")
    dma(out=wt[:], in_=W1_d[:, :])
    nc.vector.tensor_copy(out=W1b[:], in_=wt[:])
    for f in range(2):
        wt2 = wtmp_pool.tile([P, HID], f32, tag="wt")
        dma(out=wt2[:], in_=W2_d[f * P:(f + 1) * P, :])
        nc.vector.tensor_copy(out=W2s[f][:], in_=wt2[:])
        dma(out=W3s[f][:], in_=W3_d[f * P:(f + 1) * P, :])
        dma(out=Wfs[f][:], in_=Wf_d[f * P:(f + 1) * P, :])
    for brow, bd in ((b1r, b1_d), (b2r, b2_d), (b3r, b3_d)):
        wtb = wtmp_pool.tile([1, HID], f32, tag="wtb")
        dma(out=wtb[:], in_=bd[:, :])
        nc.vector.tensor_copy(out=brow[:], in_=wtb[:])
    wtmp_pool.release()

    def _early_out(tag, src_ap):
        with tc.tile_pool(name="eo_" + tag, bufs=1) as eo:
            z = eo.tile([G, NCLS], f32, tag="z", name="z_" + tag)
            nc.vector.tensor_copy(out=z[:], in_=src_ap)
            dma(out=out_d[:, :], in_=z[:])

    # ============== edge pass machinery ==============
    def edge_pass(pass_no, table_lo, table_hi, elem, finalize):
        """Gather chunks + one-hot matmuls; finalize(j, psum_getter) per bin.

        pass_no 1: transposed accumulation (psum [P,P]; lhsT=gathered, rhs=oh)
        pass 2/3: scatter accumulation (psum [P,HID]; lhsT=oh, rhs=gathered)
        """
        transposed = (pass_no == 1)
        with tc.tile_pool(name=f"ck{pass_no}", bufs=8) as ckp, \
             tc.tile_pool(name=f"oh{pass_no}", bufs=8) as ohp_, \
             tc.tile_pool(name=f"ps{pass_no}", bufs=3, space="PSUM") as psp:
            chunks = {}   # (stream, c) -> (ck_tile, oh_tile)

            def issue_chunk(stream, c):
                if (stream, c) in chunks:
                    return
                ntile = min(CT, (NTL if stream == 'L' else NTH) - c * CT)
                nidx = ntile * P
                ck = ckp.tile([P, CT, elem], bf16, tag="ck",
                              name=f"ck{pass_no}_{stream}{c}")
                oh = ohp_.tile([P, CT, P], bf16, tag="oh",
                               name=f"oh{pass_no}_{stream}{c}")
                idx_t = idxL if stream == 'L' else idxH
                dst_t = dstL if stream == 'L' else dstH
                tab = table_lo if stream == 'L' else table_hi
                c0 = c * CT * P // 16
                nc.gpsimd.dma_gather(
                    ck[:, 0:ntile, :], tab, idx_t[:, c0:c0 + nidx // 16],
                    nidx, nidx, elem)
                t0 = c * CT
                for q0 in range(0, ntile, 4):
                    qn = min(4, ntile - q0)
                    nc.vector.tensor_tensor(
                        out=oh[:, q0:q0 + qn, :],
                        in0=dst_t[:, t0 + q0:t0 + q0 + qn].to_broadcast([P, qn, P]),
                        in1=iota4[:, 0:qn, :], op=OP.is_equal)
                chunks[(stream, c)] = (ck, oh)
                return

            for j in range(CH):
                ps = psp.tile([P, P if transposed else HID], f32, tag="agg",
                              name=f"agg{pass_no}_{j}")
                first = True
                for stream, nt, T in (('L', NTL, TL), ('H', NTH, TH)):
                    for t in range(j * T, (j + 1) * T):
                        c, slot = t // CT, t % CT
                        issue_chunk(stream, c)
                        ck, oh = chunks[(stream, c)]
                        if transposed:
                            nc.tensor.matmul(out=ps[:], lhsT=ck[:, slot, :],
                                             rhs=oh[:, slot, :],
                                             start=first, stop=False)
                        else:
                            nc.tensor.matmul(out=ps[:], lhsT=oh[:, slot, :],
                                             rhs=ck[:, slot, :],
                                             start=first, stop=False)
                        first = False
                finalize(j, ps)

    # ---- pass 1: hT = relu((A xs)[j] @ W1 + b1), produced transposed ----
    def fin1(j, ps):
        with tc.tile_pool(name="f1", bufs=3) as fp, \
             tc.tile_pool(name="f1p", bufs=2, space="PSUM") as fpp:
            # self-loop: += xsT column block; closes accumulation
            nc.tensor.matmul(out=ps[:], lhsT=eye[:],
                             rhs=xsT_sb[:, j * P:(j + 1) * P],
                             start=False, stop=True)
            t1 = fp.tile([P, P], bf16, tag="t1")
            nc.vector.tensor_tensor(
                out=t1[:], in0=ps[:],
                in1=disb_sb[:, j * P:(j + 1) * P],
                op=OP.mult)
            for f, hT in enumerate((hT0, hT1)):
                pst = fpp.tile([P, P], f32, tag="pst")
                nc.tensor.matmul(out=pst[:], lhsT=W1b[:, f * P:(f + 1) * P],
                                 rhs=t1[:], start=True, stop=False)
                nc.tensor.matmul(out=pst[:], lhsT=b1r[0:1, f * P:(f + 1) * P],
                                 rhs=maskrow[0:1, j * P:(j + 1) * P],
                                 start=False, stop=True)
                nc.scalar.activation(out=hT[:, j * P:(j + 1) * P], in_=pst[:],
                                     func=AF.Relu)

    xsp = tc.alloc_tile_pool(name="xsp", bufs=1)
    xsT_sb = xsp.tile([P, SL], bf16, name="xsT_sb")
    disb_sb = xsp.tile([P, SL], f32, name="disb_sb")
    dma(out=xsT_sb[:], in_=xsT_d[:, :])
    dma(out=disb_sb[:], in_=disb_d[:, :])

    edge_pass(1, xs_d[:, :], xs_d[HALF:S, :], IN, fin1)
    xsp.release()
    if STOP == 'P1':
        _early_out('P1', hT0[0:G, 0:NCLS]); ctx.close(); return

    # ---- make_hs: hs = dis * (hT.T @ W) [+ shift row], store + allgather ----
    def make_hs(Ws, hsloc_dram, hs_dram, add_r):
        with tc.tile_pool(name="pH", bufs=4) as pH, \
             tc.tile_pool(name="pHp", bufs=2, space="PSUM") as pHp:
            for j in range(CH):
                ps = pHp.tile([P, HID], f32, tag="hs")
                nc.tensor.matmul(out=ps[:], lhsT=hT0[:, j * P:(j + 1) * P],
                                 rhs=Ws[0][:], start=True, stop=False)
                nc.tensor.matmul(out=ps[:], lhsT=hT1[:, j * P:(j + 1) * P],
                                 rhs=Ws[1][:], start=False, stop=not add_r)
                if add_r:
                    nc.tensor.matmul(out=ps[:], lhsT=onesrow[:],
                                     rhs=rrow[:], start=False, stop=True)
                nc.scalar.activation(out=hsloc[:, j * HID:(j + 1) * HID], in_=ps[:],
                                     func=AF.Copy, scale=disloc[:, j:j + 1])
                dma(out=hsloc_dram[j * P:(j + 1) * P, :],
                    in_=hsloc[:, j * HID:(j + 1) * HID])
        nc.gpsimd.collective_compute(
            "AllGather", mybir.AluOpType.bypass, replica_groups=GRP,
            ins=[hsloc_dram.ap().opt()], outs=[hs_dram.ap().opt()])

    make_hs(W2s, hs2loc_d, hs2_d, add_r=False)
    if STOP == 'H2':
        _early_out('H2', hsloc[0:G, 0:NCLS]); ctx.close(); return

    # ---- pass 2: h2 = relu(dis*(agg + self + invdis x b2)); hT + BN1 stats ----
    def fin23(j, ps, hso, brow, post):
        # self-loop rows + bias close the accumulation
        nc.tensor.matmul(out=ps[:], lhsT=eye[:],
                         rhs=hso[:, j * HID:(j + 1) * HID], start=False, stop=False)
        nc.tensor.matmul(out=ps[:], lhsT=invdis[0:1, j * P:(j + 1) * P],
                         rhs=brow[:], start=False, stop=True)
        post(j, ps)

    def post2(j, ps):
        with tc.tile_pool(name="f2", bufs=3) as fp, \
             tc.tile_pool(name="f2p", bufs=2, space="PSUM") as fpp:
            t2 = fp.tile([P, HID], bf16, tag="t2")
            nc.scalar.activation(out=t2[:], in_=ps[:], func=AF.Relu,
                                 scale=disloc[:, j:j + 1])
            for f, hT in enumerate((hT0, hT1)):
                pst = fpp.tile([P, P], bf16, tag="pst")
                nc.tensor.transpose(out=pst[:], in_=t2[:, f * P:(f + 1) * P],
                                    identity=eye[:])
                nc.scalar.activation(out=hT[:, j * P:(j + 1) * P], in_=pst[:],
                                     func=AF.Copy)

    edge_pass(2, hs2_d[:, :], hs2_d[HALF:S, :], HID,
              lambda j, ps: fin23(j, ps, hsloc, b2r, post2))
    if STOP == 'P2':
        _early_out('P2', hT0[0:G, 0:NCLS]); ctx.close(); return

    # ---- BN1 stats from hT (bf16) -> allreduce -> scale1/shift1 ----
    with tc.tile_pool(name="pS", bufs=2) as pS:
        st = pS.tile([P, 4], f32, tag="st")
        sq = pS.tile([P, SL], bf16, tag="sq")
        for f, hT in enumerate((hT0, hT1)):
            nc.vector.tensor_reduce(out=st[:, f:f + 1], in_=hT[:, :],
                                    axis=mybir.AxisListType.X, op=OP.add)
            nc.scalar.activation(out=sq[:], in_=hT[:, :], func=AF.Square)
            nc.vector.tensor_reduce(out=st[:, 2 + f:3 + f], in_=sq[:, :],
                                    axis=mybir.AxisListType.X, op=OP.add)
        dma(out=st1_in[:, :], in_=st[:])
    nc.gpsimd.collective_compute(
        "AllReduce", mybir.AluOpType.add, replica_groups=GRP,
        ins=[st1_in.ap().opt()], outs=[st1_out.ap().opt()])
    dma(out=stats1[:], in_=st1_out[:, :])

    def bn_fold(stats_sums, stats_sqs, gc, bec, scale_t, shift_t, pool):
        mu = pool.tile([P, 2], f32, tag="mu")
        var = pool.tile([P, 2], f32, tag="var")
        tmp = pool.tile([P, 2], f32, tag="tmp")
        nc.vector.tensor_scalar_mul(out=mu[:], in0=stats_sums, scalar1=1.0 / N)
        nc.vector.tensor_scalar_mul(out=var[:], in0=stats_sqs, scalar1=1.0 / N)
        nc.vector.tensor_tensor(out=tmp[:], in0=mu[:], in1=mu[:], op=OP.mult)
        nc.vector.tensor_tensor(out=var[:], in0=var[:], in1=tmp[:], op=OP.subtract)
        nc.scalar.activation(out=tmp[:], in_=var[:], func=AF.Sqrt, bias=epscol[:])
        nc.vector.reciprocal(out=tmp[:], in_=tmp[:])
        nc.vector.tensor_tensor(out=scale_t[:], in0=gc[:], in1=tmp[:], op=OP.mult)
        nc.vector.tensor_tensor(out=tmp[:], in0=mu[:], in1=scale_t[:], op=OP.mult)
        nc.vector.tensor_tensor(out=shift_t[:], in0=bec[:], in1=tmp[:], op=OP.subtract)

    with tc.tile_pool(name="pB", bufs=1) as pB, \
         tc.tile_pool(name="pBp", bufs=1, space="PSUM") as pBp:
        bn_fold(stats1[:, 0:2], stats1[:, 2:4], g1c, be1c, scale1, shift1, pB)
        # W3' = scale1 (*) W3 rows; rrow = shift1 @ W3
        psr_ = pBp.tile([1, HID], f32, tag="rr")
        for f in range(2):
            w3f = pB.tile([P, HID], f32, tag="w3f")
            nc.vector.tensor_scalar_mul(out=w3f[:], in0=W3s[f][:],
                                        scalar1=scale1[:, f:f + 1])
            nc.vector.tensor_copy(out=W3p[f][:], in_=w3f[:])
            nc.tensor.matmul(out=psr_[:], lhsT=shift1[:, f:f + 1], rhs=W3s[f][:],
                             start=(f == 0), stop=(f == 1))
        nc.vector.tensor_copy(out=rrow[:], in_=psr_[:])

    make_hs(W3p, hs3loc_d, hs3_d, add_r=True)
    if STOP == 'H3':
        _early_out('H3', hsloc[0:G, 0:NCLS]); ctx.close(); return

    # ---- pass 3: h3 = relu(...); fused pooling + BN2 sq-stats ----
    with tc.tile_pool(name="pGp", bufs=1, space="PSUM") as pGp:
        poolT = [pGp.tile([P, G], f32, tag=f"pool_{f}", name=f"pool_{f}")
                 for f in range(2)]
        s2p = [pGp.tile([P, 1], f32, tag=f"gs2_{f}", name=f"gs2_{f}")
               for f in range(2)]

        def post3(j, ps):
            with tc.tile_pool(name="f3", bufs=3) as fp:
                t3 = fp.tile([P, HID], bf16, tag="t3")
                nc.scalar.activation(out=t3[:], in_=ps[:], func=AF.Relu,
                                     scale=disloc[:, j:j + 1])
                ohg = fp.tile([P, G], bf16, tag="ohg")
                nc.vector.tensor_tensor(
                    out=ohg[:], in0=batchloc[:, j:j + 1].to_broadcast([P, G]),
                    in1=iotaG[:], op=OP.is_equal)
                sqt = fp.tile([P, HID], f32, tag="sqt")
                nc.scalar.activation(out=sqt[:], in_=t3[:], func=AF.Square)
                for f in range(2):
                    nc.tensor.matmul(out=poolT[f][:],
                                     lhsT=t3[:, f * P:(f + 1) * P], rhs=ohg[:],
                                     start=(j == 0), stop=(j == CH - 1))
                    nc.tensor.matmul(out=s2p[f][:],
                                     lhsT=sqt[:, f * P:(f + 1) * P], rhs=onescol[:],
                                     start=(j == 0), stop=(j == CH - 1))

        edge_pass(3, hs3_d[:, :], hs3_d[HALF:S, :], HID,
                  lambda j, ps: fin23(j, ps, hsloc, b3r, post3))

        # ---- pack pooled sums + BN2 stats -> one allreduce ----
        with tc.tile_pool(name="pG2", bufs=1) as pG2:
            arp = pG2.tile([P, 132], f32, tag="arp")
            for f in range(2):
                nc.vector.tensor_copy(out=arp[:, f * G:(f + 1) * G], in_=poolT[f][:])
                nc.vector.tensor_reduce(out=arp[:, 128 + f:129 + f],
                                        in_=poolT[f][:],
                                        axis=mybir.AxisListType.X, op=OP.add)
                nc.vector.tensor_copy(out=arp[:, 130 + f:131 + f], in_=s2p[f][:])
            dma(out=ar2_in[:, :], in_=arp[:])
    nc.gpsimd.collective_compute(
        "AllReduce", mybir.AluOpType.add, replica_groups=GRP,
        ins=[ar2_in.ap().opt()], outs=[ar2_out.ap().opt()])
    dma(out=ar2[:], in_=ar2_out[:, :])

    # ---- fold BN2 into Wf, final matmul ----
    with tc.tile_pool(name="pF", bufs=1) as pF, \
         tc.tile_pool(name="pFp", bufs=1, space="PSUM") as pFp:
        bn_fold(ar2[:, 128:130], ar2[:, 130:132], g2c, be2c, scale2, shift2, pF)
        psb = pFp.tile([1, NCLS], f32, tag="psb")
        for f in range(2):
            nc.vector.tensor_scalar_mul(out=Wfp[f][:], in0=Wfs[f][:],
                                        scalar1=scale2[:, f:f + 1])
            nc.tensor.matmul(out=psb[:], lhsT=shift2[:, f:f + 1], rhs=Wfs[f][:],
                             start=(f == 0), stop=False)
        nc.tensor.matmul(out=psb[:], lhsT=onescol[0:1, 0:1], rhs=bfr[:],
                         start=False, stop=True)
        nc.vector.tensor_copy(out=bfp[:], in_=psb[:])

        pso = pFp.tile([G, NCLS], f32, tag="pso")
        for f in range(2):
            nc.tensor.matmul(out=pso[:], lhsT=ar2[:, f * G:(f + 1) * G],
                             rhs=Wfp[f][:], start=(f == 0), stop=False)
        nc.tensor.matmul(out=pso[:], lhsT=cntrow[:], rhs=bfp[:],
                         start=False, stop=True)
        osb = pF.tile([G, NCLS], f32, tag="osb")
        nc.vector.tensor_scalar_mul(out=osb[:], in0=pso[:], scalar1=invcnt[:])
        dma(out=out_d[:, :], in_=osb[:])

    ctx.close()


def kernel(x, edge_index, batch, W1, b1, W2, b2, W3, b3, g1, be1, g2, be2, Wf, bf):
    global LAST_EXEC_NS, LAST_RESULTS
    from concourse import bacc, tile
    from concourse.bass_utils import run_bass_kernel_spmd

    x = np.asarray(x)
    edge_index = np.asarray(edge_index)
    batch = np.asarray(batch)

    per_core, shared, TL, TH = _preprocess(x, edge_index, batch)

    nc = bacc.Bacc("TRN2", target_bir_lowering=False, debug=False,
                   num_devices=CORES)
    with tile.TileContext(nc) as tc:
        _build(nc, tc, TL, TH)
    nc.compile()

    def col2(v):
        return np.ascontiguousarray(np.asarray(v, F32).reshape(2, P).T)

    base = dict(
        xs=shared["xs"], iota4=shared["iota4"], iotaG=shared["iotaG"],
        eye=shared["eye"], onescol=shared["onescol"], onesrow=shared["onesrow"],
        cntrow=shared["cntrow"], invcntcol=shared["invcntcol"],
        W1=np.asarray(W1, F32), W2=np.asarray(W2, F32), W3=np.asarray(W3, F32),
        Wf=np.asarray(Wf, F32),
        b1=np.asarray(b1, F32).reshape(1, HID), b2=np.asarray(b2, F32).reshape(1, HID),
        b3=np.asarray(b3, F32).reshape(1, HID), bf=np.asarray(bf, F32).reshape(1, NCLS),
        g1c=col2(g1), be1c=col2(be1), g2c=col2(g2), be2c=col2(be2),
    )
    in_maps = []
    for c in range(CORES):
        m = dict(base)
        m.update(per_core[c])
        in_maps.append(m)

    res = run_bass_kernel_spmd(nc, in_maps, core_ids=list(range(CORES)))
    LAST_EXEC_NS = res.exec_time_ns
    LAST_RESULTS = res
    return np.asarray(res.results[0]["out"], F32)


# revision 12
# speedup vs baseline: 3.5793x; 2.3404x over previous
"""GCN model (3x GCNConv + 2x BatchNorm + global mean pool + linear) on 8 TRN2 cores.

Strategy (v2):
- Host: bin-pack nodes into 392 bins of <=128 slots balancing per-bin in-edge
  counts; remap node ids to (bin, slot); 49 bins per core.  Per bin, edges are
  split by src slot into LOW (<32768) and HIGH streams so gather indices fit
  int16; each segment padded to 128-edge tiles with fixed per-bin tile counts
  (TL/TH = global maxima) so the SPMD instruction stream is core-uniform.
- Device (SPMD x8): edge aggregation via batched SWDGE dma_gather (1024 rows
  per instruction, 8x fewer fixed-overhead hits than 128-row indirect DMA)
  + one-hot (iota compare, 4 tiles per DVE op) matmuls accumulating in PSUM.
- Conv1 is computed as (A x) @ W1: gathers 128-dim dis-scaled x rows (host
  param), accumulates transposed so hT is produced with no transposes and no
  device-side x@W1 pre-pass.
- Self-loop contributions added via eye-matmul on resident local rows (not
  gathered).  Conv biases as K=1 rank-1 matmuls into PSUM.  BatchNorms folded
  into following matmul weights + rank-1 shift.  Cross-core: bf16 AllGather
  (Shared outputs) of the node-feature table between layers, small AllReduce
  for BN stats / pooled sums.
"""

import os

import numpy as np
import ml_dtypes

N = 50000
E = 800000
IN = 128
HID = 256
G = 64
NCLS = 10
BN_EPS = 1e-5

P = 128
CORES = 8
BINS = 392          # global 128-slot bins (392*128 = 50176 slots)
CH = BINS // CORES  # 49 bins per core
SL = CH * P         # 6272 slots per core
S = BINS * P        # 50176 total slots
HALF = 32768        # int16 index limit for gather tables
CT = 8              # tiles per gather chunk (1024 idxs = 64 desc/engine cap)

F32 = np.float32
BF16 = ml_dtypes.bfloat16

LAST_EXEC_NS = None
LAST_RESULTS = None


def _wrap16(idx):
    """[n] int -> [128, n//16] int16: element e at [e%16, e//16], replicated x8."""
    n = len(idx)
    a = np.asarray(idx, np.int16).reshape(n // 16, 16).T
    return np.ascontiguousarray(np.tile(a, (8, 1)))


def _preprocess(x, edge_index, batch):
    import heapq

    src = edge_index[0].astype(np.int64)
    dst = edge_index[1].astype(np.int64)
    deg = 1.0 + np.bincount(dst, minlength=N).astype(np.float64)
    dis = (1.0 / np.sqrt(deg)).astype(F32)

    # ---- bin-pack nodes into BINS bins (<=128 nodes each), balancing edges ----
    w = np.bincount(dst, minlength=N).astype(np.int64)
    order = np.argsort(-w, kind="stable")
    heap = [(0, b) for b in range(BINS)]
    heapq.heapify(heap)
    count = np.zeros(BINS, np.int64)
    new_id = np.empty(N, np.int64)
    for n in order:
        while True:
            load, b = heapq.heappop(heap)
            if count[b] < P:
                break
        new_id[n] = b * P + count[b]
        count[b] += 1
        heapq.heappush(heap, (load + int(w[n]), b))

    # ---- edges grouped by dst bin, split by src half, sorted by src ----
    es = new_id[src]
    ed = new_id[dst]
    bin_e = ed // P
    # order: (bin, is_high, src)
    key = bin_e * 4 * S + (es >= HALF) * 2 * S + es
    o = np.argsort(key, kind="stable")
    es, ed, bin_e = es[o], ed[o], bin_e[o]
    islo = es < HALF
    nlo = np.bincount(bin_e[islo], minlength=BINS)
    nhi = np.bincount(bin_e[~islo], minlength=BINS)
    TL = int(np.ceil(nlo.max() / P))
    TH = int(np.ceil(nhi.max() / P))

    # per-bin padded segments
    capL, capH = TL * P, TH * P
    idxL = np.zeros((BINS, capL), np.int64)
    dstL = np.full((BINS, capL), 255.0, F32)
    idxH = np.zeros((BINS, capH), np.int64)
    dstH = np.full((BINS, capH), 255.0, F32)
    starts = np.concatenate([[0], np.cumsum(nlo + nhi)[:-1]])
    for b in range(BINS):
        s0 = starts[b]
        lo_n, hi_n = nlo[b], nhi[b]
        idxL[b, :lo_n] = es[s0:s0 + lo_n]
        dstL[b, :lo_n] = (ed[s0:s0 + lo_n] % P).astype(F32)
        idxH[b, :hi_n] = es[s0 + lo_n:s0 + lo_n + hi_n] - HALF
        dstH[b, :hi_n] = (ed[s0 + lo_n:s0 + lo_n + hi_n] % P).astype(F32)

    # ---- per-slot arrays ----
    slot_dis = np.zeros(S, F32)
    slot_dis[new_id] = dis
    slot_invdis = np.zeros(S, F32)
    slot_invdis[new_id] = np.sqrt(deg).astype(F32)
    slot_mask = np.zeros(S, F32)
    slot_mask[new_id] = 1.0
    slot_batch = np.full(S, 255.0, F32)
    slot_batch[new_id] = batch.astype(F32)

    # dis-scaled x rows, slot order: gather table for pass 1
    xs = np.zeros((S, IN), F32)
    xs[new_id] = x * dis[:, None]
    xs = xs.astype(BF16)

    cnts = np.bincount(batch.astype(np.int64), minlength=G).astype(F32)
    cnts = np.maximum(cnts, 1.0)

    per_core = []
    for c in range(CORES):
        b0, b1 = c * CH, (c + 1) * CH
        s0, s1 = c * SL, (c + 1) * SL
        per_core.append(
            dict(
                idxL=_wrap16(idxL[b0:b1].reshape(-1)),        # [128, CH*TL*8]
                idxH=_wrap16(idxH[b0:b1].reshape(-1)),
                dstL=np.ascontiguousarray(
                    dstL[b0:b1].reshape(CH * TL, P).T).astype(BF16),  # [128, CH*TL]
                dstH=np.ascontiguousarray(
                    dstH[b0:b1].reshape(CH * TH, P).T).astype(BF16),
                disloc=np.ascontiguousarray(slot_dis[s0:s1].reshape(CH, P).T),  # [128, CH] f32
                disrow=np.ascontiguousarray(slot_dis[s0:s1].reshape(1, SL)),    # [1, SL] f32
                invdis=np.ascontiguousarray(slot_invdis[s0:s1].reshape(1, SL)).astype(BF16),
                maskrow=np.ascontiguousarray(slot_mask[s0:s1].reshape(1, SL)).astype(BF16),
                batchloc=np.ascontiguousarray(slot_batch[s0:s1].reshape(CH, P).T).astype(BF16),
                xsT=np.ascontiguousarray(xs[s0:s1].astype(F32).T).astype(BF16),  # [128, SL]
                disb=np.ascontiguousarray(
                    np.tile(slot_dis[s0:s1].reshape(1, SL), (P, 1))),  # [128, SL] f32
            )
        )

    iota4 = np.tile(np.arange(P, dtype=F32), (P, 4, 1)).astype(BF16)  # [128,4,128]
    iotaG = np.tile(np.arange(G, dtype=F32), (P, 1)).astype(BF16)     # [128,64]
    shared = dict(
        xs=xs,
        iota4=iota4,
        iotaG=iotaG,
        eye=np.eye(P, dtype=F32).astype(BF16),
        onescol=np.ones((P, 1), F32),
        onesrow=np.ones((1, P), F32).astype(BF16),
        cntrow=cnts.reshape(1, G),
        invcntcol=(1.0 / cnts).reshape(G, 1),
    )
    return per_core, shared, TL, TH


def _build(nc, tc, TL, TH):
    from concourse import bass, mybir
    from concourse import library_config
    STOP = os.environ.get('K_STOP', '')

    f32 = mybir.dt.float32
    bf16 = mybir.dt.bfloat16
    i16 = mybir.dt.int16
    AF = mybir.ActivationFunctionType
    OP = mybir.AluOpType

    NTL, NTH = CH * TL, CH * TH      # stream tiles per core per pass
    NCL = (NTL + CT - 1) // CT       # L chunks
    NCH = (NTH + CT - 1) // CT

    # ---------------- parameters ----------------
    def par(name, shape, dt):
        return nc.declare_dram_parameter(name, list(shape), dt, isOutput=False)

    xs_d = par("xs", (S, IN), bf16)
    idxL_d = par("idxL", (P, NTL * P // 16), i16)
    idxH_d = par("idxH", (P, NTH * P // 16), i16)
    dstL_d = par("dstL", (P, NTL), bf16)
    dstH_d = par("dstH", (P, NTH), bf16)
    disloc_d = par("disloc", (P, CH), f32)
    disrow_d = par("disrow", (1, SL), f32)
    invdis_d = par("invdis", (1, SL), bf16)
    maskrow_d = par("maskrow", (1, SL), bf16)
    batch_d = par("batchloc", (P, CH), bf16)
    xsT_d = par("xsT", (P, SL), bf16)
    disb_d = par("disb", (P, SL), f32)
    iota4_d = par("iota4", (P, 4, P), bf16)
    iotaG_d = par("iotaG", (P, G), bf16)
    eye_d = par("eye", (P, P), bf16)
    onescol_d = par("onescol", (P, 1), f32)
    onesrow_d = par("onesrow", (1, P), bf16)
    cntrow_d = par("cntrow", (1, G), f32)
    invcnt_d = par("invcntcol", (G, 1), f32)
    W1_d = par("W1", (IN, HID), f32)
    W2_d = par("W2", (HID, HID), f32)
    W3_d = par("W3", (HID, HID), f32)
    Wf_d = par("Wf", (HID, NCLS), f32)
    b1_d = par("b1", (1, HID), f32)
    b2_d = par("b2", (1, HID), f32)
    b3_d = par("b3", (1, HID), f32)
    bf_d = par("bf", (1, NCLS), f32)
    g1_d = par("g1c", (P, 2), f32)
    be1_d = par("be1c", (P, 2), f32)
    g2_d = par("g2c", (P, 2), f32)
    be2_d = par("be2c", (P, 2), f32)
    out_d = nc.declare_dram_parameter("out", [G, NCLS], f32, isOutput=True)

    # ---------------- device DRAM ----------------
    hs2loc_d = nc.dram_tensor("hs2loc", [SL, HID], bf16)
    hs3loc_d = nc.dram_tensor("hs3loc", [SL, HID], bf16)
    hs2_d = nc.dram_tensor("hs2", [S, HID], bf16, addr_space="Shared")
    hs3_d = nc.dram_tensor("hs3", [S, HID], bf16, addr_space="Shared")
    st1_in = nc.dram_tensor("st1_in", [P, 4], f32)
    st1_out = nc.dram_tensor("st1_out", [P, 4], f32, addr_space="Shared")
    ar2_in = nc.dram_tensor("ar2_in", [P, 132], f32)
    ar2_out = nc.dram_tensor("ar2_out", [P, 132], f32, addr_space="Shared")

    GRP = [list(range(CORES))]

    nc.gpsimd.load_library(library_config.mlp)

    # ---------------- resident SBUF ----------------
    import contextlib

    ctx = contextlib.ExitStack()
    res = ctx.enter_context(tc.tile_pool(name="res", bufs=1))

    idxL = res.tile([P, NTL * P // 16], i16)
    idxH = res.tile([P, NTH * P // 16], i16)
    dstL = res.tile([P, NTL], bf16)
    dstH = res.tile([P, NTH], bf16)
    disloc = res.tile([P, CH], f32)
    disrow = res.tile([1, SL], f32)
    invdis = res.tile([1, SL], bf16)
    maskrow = res.tile([1, SL], bf16)
    batchloc = res.tile([P, CH], bf16)
    iota4 = res.tile([P, 4, P], bf16)
    iotaG = res.tile([P, G], bf16)
    eye = res.tile([P, P], bf16)
    onescol = res.tile([P, 1], f32)
    onesrow = res.tile([1, P], bf16)
    cntrow = res.tile([1, G], f32)
    invcnt = res.tile([G, 1], f32)
    hT0 = res.tile([P, SL], bf16)             # transposed features, feat 0-127
    hT1 = res.tile([P, SL], bf16)             # feat 128-255
    hsloc = res.tile([P, CH * HID], bf16)     # resident local hs rows (self-loop)
    W1b = res.tile([IN, HID], bf16)
    W2s = [res.tile([P, HID], bf16, tag=f"w2_{f}", name=f"w2_{f}") for f in range(2)]
    W3s = [res.tile([P, HID], f32, tag=f"w3_{f}", name=f"w3_{f}") for f in range(2)]
    W3p = [res.tile([P, HID], bf16, tag=f"w3p_{f}", name=f"w3p_{f}") for f in range(2)]
    Wfs = [res.tile([P, NCLS], f32, tag=f"wf_{f}", name=f"wf_{f}") for f in range(2)]
    Wfp = [res.tile([P, NCLS], f32, tag=f"wfp_{f}", name=f"wfp_{f}") for f in range(2)]
    b1r = res.tile([1, HID], bf16)
    b2r = res.tile([1, HID], bf16)
    b3r = res.tile([1, HID], bf16)
    bfr = res.tile([1, NCLS], f32)
    g1c = res.tile([P, 2], f32)
    be1c = res.tile([P, 2], f32)
    g2c = res.tile([P, 2], f32)
    be2c = res.tile([P, 2], f32)
    rrow = res.tile([1, HID], bf16)
    bfp = res.tile([1, NCLS], f32)
    scale1 = res.tile([P, 2], f32)
    shift1 = res.tile([P, 2], f32)
    scale2 = res.tile([P, 2], f32)
    shift2 = res.tile([P, 2], f32)
    stats1 = res.tile([P, 4], f32)
    ar2 = res.tile([P, 132], f32)
    epscol = res.tile([P, 1], f32)
    nc.vector.memset(epscol[:], BN_EPS)

    dma = nc.sync.dma_start
    for dst_t, src_t in [
        (idxL, idxL_d), (idxH, idxH_d), (dstL, dstL_d), (dstH, dstH_d),
        (disloc, disloc_d), (disrow, disrow_d), (invdis, invdis_d),
        (maskrow, maskrow_d), (batchloc, batch_d),
        (iotaG, iotaG_d), (eye, eye_d), (onescol, onescol_d),
        (onesrow, onesrow_d), (cntrow, cntrow_d), (invcnt, invcnt_d),
        (bfr, bf_d), (g1c, g1_d), (be1c, be1_d), (g2c, g2_d), (be2c, be2_d),
    ]:
        dma(out=dst_t[:], in_=src_t[:, :])
    dma(out=iota4[:, :, :], in_=iota4_d[:, :, :])
    # weights / biases: cast f32 -> bf16 through SBUF
    wtmp_pool = tc.alloc_tile_pool(name="wtmp", bufs=2)
    wt = wtmp_pool.tile([IN, HID], f32, tag="wt")
    dma(out=wt[:], in_=W1_d[:, :])
    nc.vector.tensor_copy(out=W1b[:], in_=wt[:])
    for f in range(2):
        wt2 = wtmp_pool.tile([P, HID], f32, tag="wt")
        dma(out=wt2[:], in_=W2_d[f * P:(f + 1) * P, :])
        nc.vector.tensor_copy(out=W2s[f][:], in_=wt2[:])
        dma(out=W3s[f][:], in_=W3_d[f * P:(f + 1) * P, :])
        dma(out=Wfs[f][:], in_=Wf_d[f * P:(f + 1) * P, :])
    for brow, bd in ((b1r, b1_d), (b2r, b2_d), (b3r, b3_d)):
        wtb = wtmp_pool.tile([1, HID], f32, tag="wtb")
        dma(out=wtb[:], in_=bd[:, :])
        nc.vector.tensor_copy(out=brow[:], in_=wtb[:])
    wtmp_pool.release()

    def _early_out(tag, src_ap):
        with tc.tile_pool(name="eo_" + tag, bufs=1) as eo:
            z = eo.tile([G, NCLS], f32, tag="z", name="z_" + tag)
            nc.vector.tensor_copy(out=z[:], in_=src_ap)
            dma(out=out_d[:, :], in_=z[:])

    # ============== edge pass machinery ==============
    def edge_pass(pass_no, table_lo, table_hi, elem, finalize):
        """Gather chunks + one-hot matmuls; finalize(j, psum_getter) per bin.

        pass_no 1: transposed accumulation (psum [P,P]; lhsT=gathered, rhs=oh)
        pass 2/3: scatter accumulation (psum [P,HID]; lhsT=oh, rhs=gathered)
        """
        transposed = (pass_no == 1)
        with tc.tile_pool(name=f"ck{pass_no}", bufs=8) as ckp, \
             tc.tile_pool(name=f"oh{pass_no}", bufs=8) as ohp_, \
             tc.tile_pool(name=f"ps{pass_no}", bufs=3, space="PSUM") as psp:
            chunks = {}   # (stream, c) -> (ck_tile, oh_tile)
            qctr = [0]

            def issue_chunk(stream, c):
                if (stream, c) in chunks:
                    return
                ntile = min(CT, (NTL if stream == 'L' else NTH) - c * CT)
                nidx = ntile * P
                ck = ckp.tile([P, CT, elem], bf16, tag="ck",
                              name=f"ck{pass_no}_{stream}{c}")
                oh = ohp_.tile([P, CT, P], bf16, tag="oh",
                               name=f"oh{pass_no}_{stream}{c}")
                idx_t = idxL if stream == 'L' else idxH
                dst_t = dstL if stream == 'L' else dstH
                tab = table_lo if stream == 'L' else table_hi
                c0 = c * CT * P // 16
                nc.gpsimd.dma_gather(
                    ck[:, 0:ntile, :], tab, idx_t[:, c0:c0 + nidx // 16],
                    nidx, nidx, elem, queue_num=qctr[0] % 4)
                qctr[0] += 1
                t0 = c * CT
                for q0 in range(0, ntile, 4):
                    qn = min(4, ntile - q0)
                    nc.vector.tensor_tensor(
                        out=oh[:, q0:q0 + qn, :],
                        in0=dst_t[:, t0 + q0:t0 + q0 + qn].to_broadcast([P, qn, P]),
                        in1=iota4[:, 0:qn, :], op=OP.is_equal)
                chunks[(stream, c)] = (ck, oh)
                return

            for j in range(CH):
                ps = psp.tile([P, P if transposed else HID], f32, tag="agg",
                              name=f"agg{pass_no}_{j}")
                first = True
                for stream, nt, T in (('L', NTL, TL), ('H', NTH, TH)):
                    for t in range(j * T, (j + 1) * T):
                        c, slot = t // CT, t % CT
                        issue_chunk(stream, c)
                        ck, oh = chunks[(stream, c)]
                        if transposed:
                            nc.tensor.matmul(out=ps[:], lhsT=ck[:, slot, :],
                                             rhs=oh[:, slot, :],
                                             start=first, stop=False)
                        else:
                            nc.tensor.matmul(out=ps[:], lhsT=oh[:, slot, :],
                                             rhs=ck[:, slot, :],
                                             start=first, stop=False)
                        first = False
                finalize(j, ps)

    # ---- pass 1: hT = relu((A xs)[j] @ W1 + b1), produced transposed ----
    def fin1(j, ps):
        with tc.tile_pool(name="f1", bufs=3) as fp, \
             tc.tile_pool(name="f1p", bufs=2, space="PSUM") as fpp:
            # self-loop: += xsT column block; closes accumulation
            nc.tensor.matmul(out=ps[:], lhsT=eye[:],
                             rhs=xsT_sb[:, j * P:(j + 1) * P],
                             start=False, stop=True)
            t1 = fp.tile([P, P], bf16, tag="t1")
            nc.vector.tensor_tensor(
                out=t1[:], in0=ps[:],
                in1=disb_sb[:, j * P:(j + 1) * P],
                op=OP.mult)
            for f, hT in enumerate((hT0, hT1)):
                pst = fpp.tile([P, P], f32, tag="pst")
                nc.tensor.matmul(out=pst[:], lhsT=W1b[:, f * P:(f + 1) * P],
                                 rhs=t1[:], start=True, stop=False)
                nc.tensor.matmul(out=pst[:], lhsT=b1r[0:1, f * P:(f + 1) * P],
                                 rhs=maskrow[0:1, j * P:(j + 1) * P],
                                 start=False, stop=True)
                nc.scalar.activation(out=hT[:, j * P:(j + 1) * P], in_=pst[:],
                                     func=AF.Relu)

    xsp = tc.alloc_tile_pool(name="xsp", bufs=1)
    xsT_sb = xsp.tile([P, SL], bf16, name="xsT_sb")
    disb_sb = xsp.tile([P, SL], f32, name="disb_sb")
    dma(out=xsT_sb[:], in_=xsT_d[:, :])
    dma(out=disb_sb[:], in_=disb_d[:, :])

    edge_pass(1, xs_d[:, :], xs_d[HALF:S, :], IN, fin1)
    xsp.release()
    if STOP == 'P1':
        _early_out('P1', hT0[0:G, 0:NCLS]); ctx.close(); return

    # ---- make_hs: hs = dis * (hT.T @ W) [+ shift row], store + allgather ----
    def make_hs(Ws, hsloc_dram, hs_dram, add_r):
        with tc.tile_pool(name="pH", bufs=4) as pH, \
             tc.tile_pool(name="pHp", bufs=2, space="PSUM") as pHp:
            for j in range(CH):
                ps = pHp.tile([P, HID], f32, tag="hs")
                nc.tensor.matmul(out=ps[:], lhsT=hT0[:, j * P:(j + 1) * P],
                                 rhs=Ws[0][:], start=True, stop=False)
                nc.tensor.matmul(out=ps[:], lhsT=hT1[:, j * P:(j + 1) * P],
                                 rhs=Ws[1][:], start=False, stop=not add_r)
                if add_r:
                    nc.tensor.matmul(out=ps[:], lhsT=onesrow[:],
                                     rhs=rrow[:], start=False, stop=True)
                nc.scalar.activation(out=hsloc[:, j * HID:(j + 1) * HID], in_=ps[:],
                                     func=AF.Copy, scale=disloc[:, j:j + 1])
                dma(out=hsloc_dram[j * P:(j + 1) * P, :],
                    in_=hsloc[:, j * HID:(j + 1) * HID])
        nc.gpsimd.collective_compute(
            "AllGather", mybir.AluOpType.bypass, replica_groups=GRP,
            ins=[hsloc_dram.ap().opt()], outs=[hs_dram.ap().opt()])

    make_hs(W2s, hs2loc_d, hs2_d, add_r=False)
    if STOP == 'H2':
        _early_out('H2', hsloc[0:G, 0:NCLS]); ctx.close(); return

    # ---- pass 2: h2 = relu(dis*(agg + self + invdis x b2)); hT + BN1 stats ----
    def fin23(j, ps, hso, brow, post):
        # self-loop rows + bias close the accumulation
        nc.tensor.matmul(out=ps[:], lhsT=eye[:],
                         rhs=hso[:, j * HID:(j + 1) * HID], start=False, stop=False)
        nc.tensor.matmul(out=ps[:], lhsT=invdis[0:1, j * P:(j + 1) * P],
                         rhs=brow[:], start=False, stop=True)
        post(j, ps)

    def post2(j, ps):
        with tc.tile_pool(name="f2", bufs=3) as fp, \
             tc.tile_pool(name="f2p", bufs=2, space="PSUM") as fpp:
            t2 = fp.tile([P, HID], bf16, tag="t2")
            nc.scalar.activation(out=t2[:], in_=ps[:], func=AF.Relu,
                                 scale=disloc[:, j:j + 1])
            for f, hT in enumerate((hT0, hT1)):
                pst = fpp.tile([P, P], bf16, tag="pst")
                nc.tensor.transpose(out=pst[:], in_=t2[:, f * P:(f + 1) * P],
                                    identity=eye[:])
                nc.scalar.activation(out=hT[:, j * P:(j + 1) * P], in_=pst[:],
                                     func=AF.Copy)

    edge_pass(2, hs2_d[:, :], hs2_d[HALF:S, :], HID,
              lambda j, ps: fin23(j, ps, hsloc, b2r, post2))
    if STOP == 'P2':
        _early_out('P2', hT0[0:G, 0:NCLS]); ctx.close(); return

    # ---- BN1 stats from hT (bf16) -> allreduce -> scale1/shift1 ----
    with tc.tile_pool(name="pS", bufs=2) as pS:
        st = pS.tile([P, 4], f32, tag="st")
        sq = pS.tile([P, SL], bf16, tag="sq")
        for f, hT in enumerate((hT0, hT1)):
            nc.vector.tensor_reduce(out=st[:, f:f + 1], in_=hT[:, :],
                                    axis=mybir.AxisListType.X, op=OP.add)
            nc.scalar.activation(out=sq[:], in_=hT[:, :], func=AF.Square)
            nc.vector.tensor_reduce(out=st[:, 2 + f:3 + f], in_=sq[:, :],
                                    axis=mybir.AxisListType.X, op=OP.add)
        dma(out=st1_in[:, :], in_=st[:])
    nc.gpsimd.collective_compute(
        "AllReduce", mybir.AluOpType.add, replica_groups=GRP,
        ins=[st1_in.ap().opt()], outs=[st1_out.ap().opt()])
    dma(out=stats1[:], in_=st1_out[:, :])

    def bn_fold(stats_sums, stats_sqs, gc, bec, scale_t, shift_t, pool):
        mu = pool.tile([P, 2], f32, tag="mu")
        var = pool.tile([P, 2], f32, tag="var")
        tmp = pool.tile([P, 2], f32, tag="tmp")
        nc.vector.tensor_scalar_mul(out=mu[:], in0=stats_sums, scalar1=1.0 / N)
        nc.vector.tensor_scalar_mul(out=var[:], in0=stats_sqs, scalar1=1.0 / N)
        nc.vector.tensor_tensor(out=tmp[:], in0=mu[:], in1=mu[:], op=OP.mult)
        nc.vector.tensor_tensor(out=var[:], in0=var[:], in1=tmp[:], op=OP.subtract)
        nc.scalar.activation(out=tmp[:], in_=var[:], func=AF.Sqrt, bias=epscol[:])
        nc.vector.reciprocal(out=tmp[:], in_=tmp[:])
        nc.vector.tensor_tensor(out=scale_t[:], in0=gc[:], in1=tmp[:], op=OP.mult)
        nc.vector.tensor_tensor(out=tmp[:], in0=mu[:], in1=scale_t[:], op=OP.mult)
        nc.vector.tensor_tensor(out=shift_t[:], in0=bec[:], in1=tmp[:], op=OP.subtract)

    with tc.tile_pool(name="pB", bufs=1) as pB, \
         tc.tile_pool(name="pBp", bufs=1, space="PSUM") as pBp:
        bn_fold(stats1[:, 0:2], stats1[:, 2:4], g1c, be1c, scale1, shift1, pB)
        # W3' = scale1 (*) W3 rows; rrow = shift1 @ W3
        psr_ = pBp.tile([1, HID], f32, tag="rr")
        for f in range(2):
            w3f = pB.tile([P, HID], f32, tag="w3f")
            nc.vector.tensor_scalar_mul(out=w3f[:], in0=W3s[f][:],
                                        scalar1=scale1[:, f:f + 1])
            nc.vector.tensor_copy(out=W3p[f][:], in_=w3f[:])
            nc.tensor.matmul(out=psr_[:], lhsT=shift1[:, f:f + 1], rhs=W3s[f][:],
                             start=(f == 0), stop=(f == 1))
        nc.vector.tensor_copy(out=rrow[:], in_=psr_[:])

    make_hs(W3p, hs3loc_d, hs3_d, add_r=True)
    if STOP == 'H3':
        _early_out('H3', hsloc[0:G, 0:NCLS]); ctx.close(); return

    # ---- pass 3: h3 = relu(...); fused pooling + BN2 sq-stats ----
    with tc.tile_pool(name="pGp", bufs=1, space="PSUM") as pGp:
        poolT = [pGp.tile([P, G], f32, tag=f"pool_{f}", name=f"pool_{f}")
                 for f in range(2)]
        s2p = [pGp.tile([P, 1], f32, tag=f"gs2_{f}", name=f"gs2_{f}")
               for f in range(2)]

        def post3(j, ps):
            with tc.tile_pool(name="f3", bufs=3) as fp:
                t3 = fp.tile([P, HID], bf16, tag="t3")
                nc.scalar.activation(out=t3[:], in_=ps[:], func=AF.Relu,
                                     scale=disloc[:, j:j + 1])
                ohg = fp.tile([P, G], bf16, tag="ohg")
                nc.vector.tensor_tensor(
                    out=ohg[:], in0=batchloc[:, j:j + 1].to_broadcast([P, G]),
                    in1=iotaG[:], op=OP.is_equal)
                sqt = fp.tile([P, HID], f32, tag="sqt")
                nc.scalar.activation(out=sqt[:], in_=t3[:], func=AF.Square)
                for f in range(2):
                    nc.tensor.matmul(out=poolT[f][:],
                                     lhsT=t3[:, f * P:(f + 1) * P], rhs=ohg[:],
                                     start=(j == 0), stop=(j == CH - 1))
                    nc.tensor.matmul(out=s2p[f][:],
                                     lhsT=sqt[:, f * P:(f + 1) * P], rhs=onescol[:],
                                     start=(j == 0), stop=(j == CH - 1))

        edge_pass(3, hs3_d[:, :], hs3_d[HALF:S, :], HID,
                  lambda j, ps: fin23(j, ps, hsloc, b3r, post3))

        # ---- pack pooled sums + BN2 stats -> one allreduce ----
        with tc.tile_pool(name="pG2", bufs=1) as pG2:
            arp = pG2.tile([P, 132], f32, tag="arp")
            for f in range(2):
                nc.vector.tensor_copy(out=arp[:, f * G:(f + 1) * G], in_=poolT[f][:])
                nc.vector.tensor_reduce(out=arp[:, 128 + f:129 + f],
                                        in_=poolT[f][:],
                                        axis=mybir.AxisListType.X, op=OP.add)
                nc.vector.tensor_copy(out=arp[:, 130 + f:131 + f], in_=s2p[f][:])
            dma(out=ar2_in[:, :], in_=arp[:])
    nc.gpsimd.collective_compute(
        "AllReduce", mybir.AluOpType.add, replica_groups=GRP,
        ins=[ar2_in.ap().opt()], outs=[ar2_out.ap().opt()])
    dma(out=ar2[:], in_=ar2_out[:, :])

    # ---- fold BN2 into Wf, final matmul ----
    with tc.tile_pool(name="pF", bufs=1) as pF, \
         tc.tile_pool(name="pFp", bufs=1, space="PSUM") as pFp:
        bn_fold(ar2[:, 128:130], ar2[:, 130:132], g2c, be2c, scale2, shift2, pF)
        psb = pFp.tile([1, NCLS], f32, tag="psb")
        for f in range(2):
            nc.vector.tensor_scalar_mul(out=Wfp[f][:], in0=Wfs[f][:],
                                        scalar1=scale2[:, f:f + 1])
            nc.tensor.matmul(out=psb[:], lhsT=shift2[:, f:f + 1], rhs=Wfs[f][:],
                             start=(f == 0), stop=False)
        nc.tensor.matmul(out=psb[:], lhsT=onescol[0:1, 0:1], rhs=bfr[:],
                         start=False, stop=True)
        nc.vector.tensor_copy(out=bfp[:], in_=psb[:])

        pso = pFp.tile([G, NCLS], f32, tag="pso")
        for f in range(2):
            nc.tensor.matmul(out=pso[:], lhsT=ar2[:, f * G:(f + 1) * G],
                             rhs=Wfp[f][:], start=(f == 0), stop=False)
        nc.tensor.matmul(out=pso[:], lhsT=cntrow[:], rhs=bfp[:],
                         start=False, stop=True)
        osb = pF.tile([G, NCLS], f32, tag="osb")
        nc.vector.tensor_scalar_mul(out=osb[:], in0=pso[:], scalar1=invcnt[:])
        dma(out=out_d[:, :], in_=osb[:])

    ctx.close()


def kernel(x, edge_index, batch, W1, b1, W2, b2, W3, b3, g1, be1, g2, be2, Wf, bf):
    global LAST_EXEC_NS, LAST_RESULTS
    from concourse import bacc, tile
    from concourse.bass_utils import run_bass_kernel_spmd

    x = np.asarray(x)
    edge_index = np.asarray(edge_index)
    batch = np.asarray(batch)

    per_core, shared, TL, TH = _preprocess(x, edge_index, batch)

    nc = bacc.Bacc("TRN2", target_bir_lowering=False, debug=False,
                   num_devices=CORES, num_swdge_queues=4)
    with tile.TileContext(nc) as tc:
        _build(nc, tc, TL, TH)
    nc.compile()

    def col2(v):
        return np.ascontiguousarray(np.asarray(v, F32).reshape(2, P).T)

    base = dict(
        xs=shared["xs"], iota4=shared["iota4"], iotaG=shared["iotaG"],
        eye=shared["eye"], onescol=shared["onescol"], onesrow=shared["onesrow"],
        cntrow=shared["cntrow"], invcntcol=shared["invcntcol"],
        W1=np.asarray(W1, F32), W2=np.asarray(W2, F32), W3=np.asarray(W3, F32),
        Wf=np.asarray(Wf, F32),
        b1=np.asarray(b1, F32).reshape(1, HID), b2=np.asarray(b2, F32).reshape(1, HID),
        b3=np.asarray(b3, F32).reshape(1, HID), bf=np.asarray(bf, F32).reshape(1, NCLS),
        g1c=col2(g1), be1c=col2(be1), g2c=col2(g2), be2c=col2(be2),
    )
    in_maps = []
    for c in range(CORES):
        m = dict(base)
        m.update(per_core[c])
        in_maps.append(m)

    res = run_bass_kernel_spmd(nc, in_maps, core_ids=list(range(CORES)))
    LAST_EXEC_NS = res.exec_time_ns
    LAST_RESULTS = res
    return np.asarray(res.results[0]["out"], F32)
